# revision 12
# baseline (speedup 1.0000x reference)
"""GQA attention (B=2, S=2048, HID=2048, 32 q heads / 8 kv heads, fp32 I/O)
on 8 TRN2 NeuronCores.

Sharding: sequence-parallel. Core c owns 512 tokens of batch c//4
(cores 0-3 = batch 0, cores 4-7 = batch 1). Each core computes Q/K/V for
its own tokens; K^T and V (bf16, V with a fused ones-column per kv head
that makes the PV matmul also produce the softmax row-sums) are
all-gathered within each 4-core batch group; attention and the output
projection (bias fused as an extra contraction row) are then fully local.

All matmuls run in bf16 with fp32 PSUM accumulation (fp32 matmul is 4x
slower on the PE).  All transposes / casts / padding are done host-side
in numpy, so the NEFF sees ideally-laid-out operands.
"""

import functools
from contextlib import ExitStack

import numpy as np
import ml_dtypes

import concourse.bass as bass
import concourse.mybir as mybir
import concourse.tile as tile
from concourse import bacc
from concourse.bass_utils import run_bass_kernel_spmd

BF = mybir.dt.bfloat16
F32 = mybir.dt.float32

B, S, HID = 2, 2048, 2048
NH, NKV, HD = 32, 8, 64          # q heads, kv heads, head dim
GRP = NH // NKV                  # 4 q heads per kv head
TP = 4                           # cores per batch group
TOK = S // TP                    # 512 local tokens per core
KC = HID // 128                  # 16 contraction chunks of 128
NKC = S // 128                   # 16 key chunks of 128 (full seq)
VW = NKV * (HD + 1)              # 520: V width incl. ones columns
EXP_SCALE = float(HD) ** -0.5    # 1/8 softmax scale, fused into Exp


def build_graph():
    nc = bacc.Bacc(None, target_bir_lowering=False, debug=False, num_devices=8)

    xT = nc.declare_dram_parameter("xT", [HID, TOK], BF, isOutput=False)
    wqT = nc.declare_dram_parameter("wqT", [HID, HID], BF, isOutput=False)
    wkT = nc.declare_dram_parameter("wkT", [HID, NKV * HD], BF, isOutput=False)
    wvT = nc.declare_dram_parameter("wvT", [HID, NKV * HD], BF, isOutput=False)
    woT = nc.declare_dram_parameter("woT", [HID + 1, HID], BF, isOutput=False)
    out = nc.declare_dram_parameter("out", [TOK, HID], F32, isOutput=True)

    with tile.TileContext(nc) as tc, ExitStack() as es:
        pers = es.enter_context(tc.tile_pool(name="pers", bufs=1))
        dpool = es.enter_context(tc.tile_pool(name="dram", bufs=1, space="DRAM"))

        def T(shape, dtype, *, name, space=None, addr_space="Local"):
            pool = dpool if space == "DRAM" else pers
            return pool.tile(shape, dtype, name=name, tag=name,
                             addr_space=addr_space)

        # ---- DRAM bounce buffers for the K/V all-gather -------------------
        # cc_in rows 0..511  = local K^T  [512 kvdim, 512 tok] (cols 512.. pad)
        # cc_in rows 512..1023 = local V_aug [512 tok, 520]
        cc_in = T([2 * 512, VW], BF, space="DRAM", name="cc_in")
        cc_out = T([TP * 2 * 512, VW], BF, space="DRAM", name="cc_out")

        # ---- persistent SBUF tiles ---------------------------------------
        xk = [T([128, TOK], BF, name=f"xk{k}") for k in range(KC)]
        for k in range(KC):
            nc.sync.dma_start(out=xk[k][:, :], in_=xT[k * 128:(k + 1) * 128, :])

        wk_sb = [T([128, NKV * HD], BF, name=f"wk{k}") for k in range(KC)]
        wv_sb = [T([128, NKV * HD], BF, name=f"wv{k}") for k in range(KC)]
        for k in range(KC):
            nc.sync.dma_start(out=wk_sb[k][:, :], in_=wkT[k * 128:(k + 1) * 128, :])
            nc.sync.dma_start(out=wv_sb[k][:, :], in_=wvT[k * 128:(k + 1) * 128, :])

        # row HD (partition 64) used as the K=1 lhsT for the row-sum
        # broadcast matmul — partition-aligned with the PSUM row it feeds on
        ones64 = T([HD + 1, 64], BF, name="ones64")
        nc.vector.memset(ones64[:, :], 1.0)
        ones128 = T([1, 128], BF, name="ones128")
        nc.vector.memset(ones128[:, :], 1.0)

        # =============== phase A: local K^T and V_aug, then all-gather ====
        with tc.tile_pool(name="accA", bufs=2, space="PSUM") as accA, \
             tc.tile_pool(name="stgA", bufs=2) as stgA:
            # K^T local: [512 kvdim, 512 tok] = Wk @ x^T
            for m in range(4):
                ps = accA.tile([128, TOK], F32, tag="acc")
                for k in range(KC):
                    nc.tensor.matmul(
                        out=ps[:, :],
                        lhsT=wk_sb[k][:, m * 128:(m + 1) * 128],
                        rhs=xk[k][:, :],
                        start=(k == 0), stop=(k == KC - 1))
                st = stgA.tile([128, VW], BF, tag="stg")
                nc.vector.memset(st[:, TOK:VW], 0.0)
                nc.vector.tensor_copy(out=st[:, 0:TOK], in_=ps[:, :])
                nc.sync.dma_start(out=cc_in[m * 128:(m + 1) * 128, :],
                                  in_=st[:, :])
            # V_aug local: [512 tok, 520] = x @ Wv^T with ones col per kv head
            for mt in range(4):
                ps = accA.tile([128, NKV * HD], F32, tag="acc")
                for k in range(KC):
                    nc.tensor.matmul(
                        out=ps[:, :],
                        lhsT=xk[k][:, mt * 128:(mt + 1) * 128],
                        rhs=wv_sb[k][:, :],
                        start=(k == 0), stop=(k == KC - 1))
                va = stgA.tile([128, VW], BF, tag="vstg")
                nc.vector.memset(va[:, :], 1.0)
                for kh in range(NKV):
                    nc.vector.tensor_copy(
                        out=va[:, kh * (HD + 1):kh * (HD + 1) + HD],
                        in_=ps[:, kh * HD:(kh + 1) * HD])
                nc.sync.dma_start(
                    out=cc_in[512 + mt * 128:512 + (mt + 1) * 128, :],
                    in_=va[:, :])

        nc.gpsimd.collective_compute(
            "AllGather",
            mybir.AluOpType.bypass,
            replica_groups=[[0, 1, 2, 3], [4, 5, 6, 7]],
            ins=[cc_in.opt()],
            outs=[cc_out.opt()],
        )

        # =============== phase B: Q^T (overlaps the all-gather) ===========
        # qTp[i]: [128, 512] holds two heads.  Head h lives at partition
        # base ((h//4)%2)*64 — the same base its kv head kh=h//4 occupies
        # inside the gathered K^T tiles, so scores lhsT/rhs stay aligned.
        qTp = [T([128, TOK], BF, name=f"qTp{i}") for i in range(NH // 2)]

        def q_slot(h):
            return ((h // 4) // 2) * 4 + (h % 4), ((h // 4) % 2) * 64
        with tc.tile_pool(name="wqp", bufs=16) as wqp, \
             tc.tile_pool(name="accB", bufs=2, space="PSUM") as accB, \
             tc.tile_pool(name="stgB", bufs=3) as stgB:
            wq_sb = []
            for k in range(KC):
                w = wqp.tile([128, HID], BF, tag="wq")
                nc.sync.dma_start(out=w[:, :], in_=wqT[k * 128:(k + 1) * 128, :])
                wq_sb.append(w)
            for m in range(KC):   # 16 chunks of 128 q-dims = 2 heads each
                ps = accB.tile([128, TOK], F32, tag="acc")
                for k in range(KC):
                    nc.tensor.matmul(
                        out=ps[:, :],
                        lhsT=wq_sb[k][:, m * 128:(m + 1) * 128],
                        rhs=xk[k][:, :],
                        start=(k == 0), stop=(k == KC - 1))
                st = stgB.tile([128, TOK], BF, tag="stg")
                nc.vector.tensor_copy(out=st[:, :], in_=ps[:, :])
                # route each head to its kv-parity-aligned slot via DMA
                for j in range(2):
                    h = 2 * m + j
                    i, roff = q_slot(h)
                    nc.sync.dma_start(out=qTp[i][roff:roff + 64, :],
                                      in_=st[j * 64:(j + 1) * 64, :])

        # =============== phase C: load gathered K^T / V_aug ===============
        # kTg[b*4+mt]: [128, 512] = gathered K^T rows mt*128.. of key block
        # b (kv heads 2mt at partitions 0-63, 2mt+1 at 64-127).
        # vg[c]: [128, 520] V_aug rows for key chunk c.
        kTg = [T([128, TOK], BF, name=f"kTg{i}") for i in range(NKV // 2 * TP)]
        for b in range(TP):
            for mt in range(NKV // 2):
                nc.sync.dma_start(
                    out=kTg[b * 4 + mt][:, :],
                    in_=cc_out[b * 1024 + mt * 128:b * 1024 + (mt + 1) * 128, 0:TOK])
        vg = [T([128, VW], BF, name=f"vg{c}") for c in range(NKC)]
        for c in range(NKC):
            b, lc = c // 4, c % 4
            nc.sync.dma_start(
                out=vg[c][:, :],
                in_=cc_out[b * 1024 + 512 + lc * 128:b * 1024 + 512 + (lc + 1) * 128, :])

        # =============== phase D: attention, head by head =================
        # attnT assembled as 16 pair-tiles [128, 512] (heads 2t, 2t+1)
        attnT = [T([128, TOK], BF, name=f"attnT{t}") for t in range(NH // 2)]
        with tc.tile_pool(name="sps", bufs=2, space="PSUM") as sps, \
             tc.tile_pool(name="ops", bufs=2, space="PSUM") as ops, \
             tc.tile_pool(name="bps", bufs=1, space="PSUM") as bps, \
             tc.tile_pool(name="ptp", bufs=4) as ptp, \
             tc.tile_pool(name="nrm", bufs=2) as nrm:
            for h in range(NH):
                kh = h // GRP
                qi, roff = q_slot(h)
                po = ops.tile([HD + 1, TOK], F32, tag="po")
                for c in range(NKC):
                    b, lc = c // 4, c % 4
                    ps = sps.tile([128, TOK], F32, tag="ps")
                    nc.tensor.matmul(
                        out=ps[:, :],
                        lhsT=kTg[b * 4 + kh // 2][roff:roff + 64,
                                                  lc * 128:(lc + 1) * 128],
                        rhs=qTp[qi][roff:roff + 64, :],
                        start=True, stop=True)
                    pt = ptp.tile([128, TOK], BF, tag="pt")
                    nc.scalar.activation(
                        out=pt[:, :], in_=ps[:, :],
                        func=mybir.ActivationFunctionType.Exp,
                        scale=EXP_SCALE)
                    nc.tensor.matmul(
                        out=po[:, :],
                        lhsT=vg[c][:, kh * (HD + 1):(kh + 1) * (HD + 1)],
                        rhs=pt[:, :],
                        start=(c == 0), stop=(c == NKC - 1))
                # normalize: row HD of po is the softmax denominator.
                # All scalar work stays on partition 64 (DVE can't shift
                # partitions; DMA can't read PSUM).
                lsum = nrm.tile([HD + 1, TOK], F32, tag="lsum")
                nc.vector.tensor_copy(out=lsum[HD:HD + 1, :],
                                      in_=po[HD:HD + 1, :])
                rcp = nrm.tile([HD + 1, TOK], F32, tag="rcp")
                nc.vector.reciprocal(out=rcp[HD:HD + 1, :],
                                     in_=lsum[HD:HD + 1, :])
                rcpb = nrm.tile([HD + 1, TOK], BF, tag="rcpb")
                nc.vector.tensor_copy(out=rcpb[HD:HD + 1, :],
                                      in_=rcp[HD:HD + 1, :])
                pb = bps.tile([64, TOK], F32, tag="pb")
                nc.tensor.matmul(out=pb[:, :], lhsT=ones64[HD:HD + 1, :],
                                 rhs=rcpb[HD:HD + 1, :], start=True, stop=True)
                rb = nrm.tile([64, TOK], BF, tag="rb")
                nc.vector.tensor_copy(out=rb[:, :], in_=pb[:, :])
                ah = nrm.tile([64, TOK], BF, tag="ah")
                nc.vector.tensor_mul(out=ah[:, :], in0=po[0:HD, :], in1=rb[:, :])
                # place into the pair tile (DMA shifts partitions for odd h)
                t, half = h // 2, (h % 2) * 64
                nc.sync.dma_start(out=attnT[t][half:half + 64, :], in_=ah[:, :])

        # =============== phase E: output projection + bias ================
        with tc.tile_pool(name="wop", bufs=6) as wop, \
             tc.tile_pool(name="yps", bufs=4, space="PSUM") as yps, \
             tc.tile_pool(name="ystg", bufs=3) as ystg:
            for nt in range(4):            # 4 output column blocks of 512
                wo_last = wop.tile([1, 512], BF, tag="wolast")
                nc.sync.dma_start(out=wo_last[:, :],
                                  in_=woT[HID:HID + 1, nt * 512:(nt + 1) * 512])
                pys = [yps.tile([128, 512], F32, tag="py", name=f"py{nt}_{i}")
                       for i in range(4)]
                for kc in range(KC):
                    wo_t = wop.tile([128, 512], BF, tag="wo")
                    nc.sync.dma_start(
                        out=wo_t[:, :],
                        in_=woT[kc * 128:(kc + 1) * 128, nt * 512:(nt + 1) * 512])
                    for mt in range(4):    # 4 token blocks of 128
                        nc.tensor.matmul(
                            out=pys[mt][:, :],
                            lhsT=attnT[kc][:, mt * 128:(mt + 1) * 128],
                            rhs=wo_t[:, :],
                            start=(kc == 0), stop=False)
                for mt in range(4):        # bias via ones row, K=1 matmul
                    nc.tensor.matmul(
                        out=pys[mt][:, :], lhsT=ones128[:, :],
                        rhs=wo_last[:, :], start=False, stop=True)
                    ys = ystg.tile([128, 512], F32, tag="ys")
                    nc.vector.tensor_copy(out=ys[:, :], in_=pys[mt][:, :])
                    nc.sync.dma_start(
                        out=out[mt * 128:(mt + 1) * 128, nt * 512:(nt + 1) * 512],
                        in_=ys[:, :])

    nc.finalize()
    return nc


@functools.lru_cache(maxsize=1)
def _graph():
    return build_graph()


def make_in_maps(x, Wq, Wk, Wv, Wo, bo):
    bf16 = ml_dtypes.bfloat16
    x = np.asarray(x, np.float32)
    wqT = np.ascontiguousarray(np.asarray(Wq, np.float32).T).astype(bf16)
    wkT = np.ascontiguousarray(np.asarray(Wk, np.float32).T).astype(bf16)
    wvT = np.ascontiguousarray(np.asarray(Wv, np.float32).T).astype(bf16)
    woT = np.concatenate(
        [np.asarray(Wo, np.float32).T,
         np.asarray(bo, np.float32)[None, :]], axis=0).astype(bf16)
    woT = np.ascontiguousarray(woT)
    in_maps = []
    for c in range(8):
        b, r = c // TP, c % TP
        xT_c = np.ascontiguousarray(
            x[b].T[:, r * TOK:(r + 1) * TOK]).astype(bf16)
        in_maps.append(
            {"xT": xT_c, "wqT": wqT, "wkT": wkT, "wvT": wvT, "woT": woT})
    return in_maps


def kernel(x, Wq, Wk, Wv, Wo, bo):
    nc = _graph()
    in_maps = make_in_maps(x, Wq, Wk, Wv, Wo, bo)
    res = run_bass_kernel_spmd(nc, in_maps, core_ids=list(range(8)))
    out = np.empty((B, S, HID), np.float32)
    for c in range(8):
        b, r = c // TP, c % TP
        out[b, r * TOK:(r + 1) * TOK, :] = np.asarray(
            res.results[c]["out"], np.float32)
    return out


# revision 13
# speedup vs baseline: 1.0408x; 1.0408x over previous
"""GQA attention (B=2, S=2048, HID=2048, 32 q heads / 8 kv heads, fp32 I/O)
on 8 TRN2 NeuronCores.

Sharding: sequence-parallel. Core c owns 512 tokens of batch c//4
(cores 0-3 = batch 0, cores 4-7 = batch 1). Each core computes Q/K/V for
its own tokens; K^T and V (bf16, V with a fused ones-column per kv head
that makes the PV matmul also produce the softmax row-sums) are
all-gathered within each 4-core batch group; attention and the output
projection (bias fused as an extra contraction row) are then fully local.

All matmuls run in bf16 with fp32 PSUM accumulation (fp32 matmul is 4x
slower on the PE).  All transposes / casts / padding are done host-side
in numpy, so the NEFF sees ideally-laid-out operands.
"""

import functools
from contextlib import ExitStack

import numpy as np
import ml_dtypes

import concourse.bass as bass
import concourse.mybir as mybir
import concourse.tile as tile
from concourse import bacc
from concourse.bass_utils import run_bass_kernel_spmd

BF = mybir.dt.bfloat16
F32 = mybir.dt.float32

B, S, HID = 2, 2048, 2048
NH, NKV, HD = 32, 8, 64          # q heads, kv heads, head dim
GRP = NH // NKV                  # 4 q heads per kv head
TP = 4                           # cores per batch group
TOK = S // TP                    # 512 local tokens per core
KC = HID // 128                  # 16 contraction chunks of 128
NKC = S // 128                   # 16 key chunks of 128 (full seq)
VW = NKV * (HD + 1)              # 520: V width incl. ones columns
EXP_SCALE = float(HD) ** -0.5    # 1/8 softmax scale, fused into Exp


def build_graph():
    nc = bacc.Bacc(None, target_bir_lowering=False, debug=False, num_devices=8)

    xT = nc.declare_dram_parameter("xT", [HID, TOK], BF, isOutput=False)
    wqT = nc.declare_dram_parameter("wqT", [HID, HID], BF, isOutput=False)
    wkT = nc.declare_dram_parameter("wkT", [HID, NKV * HD], BF, isOutput=False)
    wvT = nc.declare_dram_parameter("wvT", [HID, NKV * HD], BF, isOutput=False)
    woT = nc.declare_dram_parameter("woT", [HID + 1, HID], BF, isOutput=False)
    out = nc.declare_dram_parameter("out", [TOK, HID], F32, isOutput=True)

    with tile.TileContext(nc) as tc, ExitStack() as es:
        pers = es.enter_context(tc.tile_pool(name="pers", bufs=1))
        dpool = es.enter_context(tc.tile_pool(name="dram", bufs=1, space="DRAM"))

        def T(shape, dtype, *, name, space=None, addr_space="Local"):
            pool = dpool if space == "DRAM" else pers
            return pool.tile(shape, dtype, name=name, tag=name,
                             addr_space=addr_space)

        # ---- DRAM bounce buffers for the K/V all-gather -------------------
        # cc_in rows 0..511  = local K^T  [512 kvdim, 512 tok] (cols 512.. pad)
        # cc_in rows 512..1023 = local V_aug [512 tok, 520]
        cc_in = T([2 * 512, VW], BF, space="DRAM", name="cc_in")
        cc_out = T([TP * 2 * 512, VW], BF, space="DRAM", name="cc_out")

        # ---- persistent SBUF tiles ---------------------------------------
        xk = [T([128, TOK], BF, name=f"xk{k}") for k in range(KC)]
        for k in range(KC):
            nc.sync.dma_start(out=xk[k][:, :], in_=xT[k * 128:(k + 1) * 128, :])

        wk_sb = [T([128, NKV * HD], BF, name=f"wk{k}") for k in range(KC)]
        wv_sb = [T([128, NKV * HD], BF, name=f"wv{k}") for k in range(KC)]
        for k in range(KC):
            nc.sync.dma_start(out=wk_sb[k][:, :], in_=wkT[k * 128:(k + 1) * 128, :])
            nc.sync.dma_start(out=wv_sb[k][:, :], in_=wvT[k * 128:(k + 1) * 128, :])

        # row HD (partition 64) used as the K=1 lhsT for the row-sum
        # broadcast matmul — partition-aligned with the PSUM row it feeds on
        ones64 = T([HD + 1, 64], BF, name="ones64")
        nc.vector.memset(ones64[:, :], 1.0)
        ones128 = T([1, 128], BF, name="ones128")
        nc.vector.memset(ones128[:, :], 1.0)

        # =============== phase A: local K^T and V_aug, then all-gather ====
        with tc.tile_pool(name="accA", bufs=2, space="PSUM") as accA, \
             tc.tile_pool(name="stgA", bufs=2) as stgA:
            # K^T local: [512 kvdim, 512 tok] = Wk @ x^T
            for m in range(4):
                ps = accA.tile([128, TOK], F32, tag="acc")
                for k in range(KC):
                    nc.tensor.matmul(
                        out=ps[:, :],
                        lhsT=wk_sb[k][:, m * 128:(m + 1) * 128],
                        rhs=xk[k][:, :],
                        start=(k == 0), stop=(k == KC - 1))
                st = stgA.tile([128, VW], BF, tag="stg")
                nc.vector.memset(st[:, TOK:VW], 0.0)
                nc.vector.tensor_copy(out=st[:, 0:TOK], in_=ps[:, :])
                nc.sync.dma_start(out=cc_in[m * 128:(m + 1) * 128, :],
                                  in_=st[:, :])
            # V_aug local: [512 tok, 520] = x @ Wv^T with ones col per kv head
            for mt in range(4):
                ps = accA.tile([128, NKV * HD], F32, tag="acc")
                for k in range(KC):
                    nc.tensor.matmul(
                        out=ps[:, :],
                        lhsT=xk[k][:, mt * 128:(mt + 1) * 128],
                        rhs=wv_sb[k][:, :],
                        start=(k == 0), stop=(k == KC - 1))
                va = stgA.tile([128, VW], BF, tag="vstg")
                nc.vector.memset(va[:, :], 1.0)
                for kh in range(NKV):
                    nc.vector.tensor_copy(
                        out=va[:, kh * (HD + 1):kh * (HD + 1) + HD],
                        in_=ps[:, kh * HD:(kh + 1) * HD])
                nc.sync.dma_start(
                    out=cc_in[512 + mt * 128:512 + (mt + 1) * 128, :],
                    in_=va[:, :])

        nc.gpsimd.collective_compute(
            "AllGather",
            mybir.AluOpType.bypass,
            replica_groups=[[0, 1, 2, 3], [4, 5, 6, 7]],
            ins=[cc_in.opt()],
            outs=[cc_out.opt()],
        )

        # =============== phase B: Q^T (overlaps the all-gather) ===========
        # qTp[i]: [128, 512] holds two heads.  Head h lives at partition
        # base ((h//4)%2)*64 — the same base its kv head kh=h//4 occupies
        # inside the gathered K^T tiles, so scores lhsT/rhs stay aligned.
        qTp = [T([128, TOK], BF, name=f"qTp{i}") for i in range(NH // 2)]

        def q_slot(h):
            return ((h // 4) // 2) * 4 + (h % 4), ((h // 4) % 2) * 64
        with tc.tile_pool(name="wqp", bufs=16) as wqp, \
             tc.tile_pool(name="accB", bufs=2, space="PSUM") as accB, \
             tc.tile_pool(name="stgB", bufs=3) as stgB:
            wq_sb = []
            for k in range(KC):
                w = wqp.tile([128, HID], BF, tag="wq")
                nc.sync.dma_start(out=w[:, :], in_=wqT[k * 128:(k + 1) * 128, :])
                wq_sb.append(w)
            for m in range(KC):   # 16 chunks of 128 q-dims = 2 heads each
                ps = accB.tile([128, TOK], F32, tag="acc")
                for k in range(KC):
                    nc.tensor.matmul(
                        out=ps[:, :],
                        lhsT=wq_sb[k][:, m * 128:(m + 1) * 128],
                        rhs=xk[k][:, :],
                        start=(k == 0), stop=(k == KC - 1))
                st = stgB.tile([128, TOK], BF, tag="stg")
                nc.vector.tensor_copy(out=st[:, :], in_=ps[:, :])
                # route each head to its kv-parity-aligned slot via DMA
                for j in range(2):
                    h = 2 * m + j
                    i, roff = q_slot(h)
                    nc.sync.dma_start(out=qTp[i][roff:roff + 64, :],
                                      in_=st[j * 64:(j + 1) * 64, :])

        # =============== phase C: load gathered K^T / V_aug ===============
        # kTg[b*4+mt]: [128, 512] = gathered K^T rows mt*128.. of key block
        # b (kv heads 2mt at partitions 0-63, 2mt+1 at 64-127).
        # vg[c]: [128, 520] V_aug rows for key chunk c.
        kTg = [T([128, TOK], BF, name=f"kTg{i}") for i in range(NKV // 2 * TP)]
        for b in range(TP):
            for mt in range(NKV // 2):
                nc.sync.dma_start(
                    out=kTg[b * 4 + mt][:, :],
                    in_=cc_out[b * 1024 + mt * 128:b * 1024 + (mt + 1) * 128, 0:TOK])
        vg = [T([128, VW], BF, name=f"vg{c}") for c in range(NKC)]
        for c in range(NKC):
            b, lc = c // 4, c % 4
            nc.sync.dma_start(
                out=vg[c][:, :],
                in_=cc_out[b * 1024 + 512 + lc * 128:b * 1024 + 512 + (lc + 1) * 128, :])

        # =============== phase D: attention, paired heads =================
        # Heads are processed in kv-parity pairs (hA: even kv head at
        # partition base 0, hB: odd kv head at base 64).  Their score
        # matmuls are issued back-to-back into different PE row groups and
        # run concurrently.  Score PSUM tiles span two key chunks (2 banks)
        # so each Exp covers N=1024, amortizing ACT's fixed overhead.
        # attnT assembled as 16 pair-tiles [128, 512] (heads 2t, 2t+1)
        attnT = [T([128, TOK], BF, name=f"attnT{t}") for t in range(NH // 2)]
        pairs = []
        for g in range(0, NKV, 2):
            for j in range(GRP):
                pairs.append((g * GRP + j, (g + 1) * GRP + j))
        with tc.tile_pool(name="sps", bufs=2, space="PSUM") as sps, \
             tc.tile_pool(name="ops", bufs=3, space="PSUM") as ops, \
             tc.tile_pool(name="bps", bufs=1, space="PSUM") as bps, \
             tc.tile_pool(name="ptp", bufs=4) as ptp, \
             tc.tile_pool(name="nrm", bufs=2) as nrm:
            for hA, hB in pairs:
                khA, khB = hA // GRP, hB // GRP
                kt = khA // 2          # kTg row tile: khA at 0:64, khB at 64:128
                qiA, _ = q_slot(hA)
                qiB, _ = q_slot(hB)
                poA = ops.tile([HD + 1, TOK], F32, tag="po", name=f"poA{hA}")
                poB = ops.tile([HD + 1, TOK], F32, tag="po", name=f"poB{hB}")
                pts = []
                for ci in range(0, NKC, 2):
                    psA = sps.tile([128, 2 * TOK], F32, tag="ps", name=f"psA{hA}_{ci}")
                    psB = sps.tile([128, 2 * TOK], F32, tag="ps", name=f"psB{hB}_{ci}")
                    for dc in range(2):
                        c = ci + dc
                        b, lc = c // 4, c % 4
                        kts = kTg[b * 4 + kt]
                        nc.tensor.matmul(
                            out=psA[:, dc * TOK:(dc + 1) * TOK],
                            lhsT=kts[0:64, lc * 128:(lc + 1) * 128],
                            rhs=qTp[qiA][0:64, :], start=True, stop=True)
                        nc.tensor.matmul(
                            out=psB[:, dc * TOK:(dc + 1) * TOK],
                            lhsT=kts[64:128, lc * 128:(lc + 1) * 128],
                            rhs=qTp[qiB][64:128, :], start=True, stop=True)
                    ptA = ptp.tile([128, 2 * TOK], BF, tag="pt", name=f"ptA{hA}_{ci}")
                    nc.scalar.activation(
                        out=ptA[:, :], in_=psA[:, :],
                        func=mybir.ActivationFunctionType.Exp, scale=EXP_SCALE)
                    ptB = ptp.tile([128, 2 * TOK], BF, tag="pt", name=f"ptB{hB}_{ci}")
                    nc.scalar.activation(
                        out=ptB[:, :], in_=psB[:, :],
                        func=mybir.ActivationFunctionType.Exp, scale=EXP_SCALE)
                    pts.append((ci, ptA, ptB))
                for ci, ptA, ptB in pts:
                    for dc in range(2):
                        c = ci + dc
                        nc.tensor.matmul(
                            out=poA[:, :],
                            lhsT=vg[c][:, khA * (HD + 1):(khA + 1) * (HD + 1)],
                            rhs=ptA[:, dc * TOK:(dc + 1) * TOK],
                            start=(c == 0), stop=(c == NKC - 1))
                        nc.tensor.matmul(
                            out=poB[:, :],
                            lhsT=vg[c][:, khB * (HD + 1):(khB + 1) * (HD + 1)],
                            rhs=ptB[:, dc * TOK:(dc + 1) * TOK],
                            start=(c == 0), stop=(c == NKC - 1))
                for h, po in ((hA, poA), (hB, poB)):
                    # normalize: row HD of po is the softmax denominator.
                    # All scalar work stays on partition 64 (DVE can't
                    # shift partitions; DMA can't read PSUM).
                    lsum = nrm.tile([HD + 1, TOK], F32, tag="lsum", name=f"ls{h}")
                    nc.vector.tensor_copy(out=lsum[HD:HD + 1, :],
                                          in_=po[HD:HD + 1, :])
                    rcp = nrm.tile([HD + 1, TOK], F32, tag="rcp", name=f"rc{h}")
                    nc.vector.reciprocal(out=rcp[HD:HD + 1, :],
                                         in_=lsum[HD:HD + 1, :])
                    rcpb = nrm.tile([HD + 1, TOK], BF, tag="rcpb", name=f"rb{h}")
                    nc.vector.tensor_copy(out=rcpb[HD:HD + 1, :],
                                          in_=rcp[HD:HD + 1, :])
                    pb = bps.tile([64, TOK], F32, tag="pb", name=f"pb{h}")
                    nc.tensor.matmul(out=pb[:, :], lhsT=ones64[HD:HD + 1, :],
                                     rhs=rcpb[HD:HD + 1, :], start=True,
                                     stop=True)
                    rb = nrm.tile([64, TOK], BF, tag="rbb", name=f"rbb{h}")
                    nc.vector.tensor_copy(out=rb[:, :], in_=pb[:, :])
                    ah = nrm.tile([64, TOK], BF, tag="ah", name=f"ah{h}")
                    nc.vector.tensor_mul(out=ah[:, :], in0=po[0:HD, :],
                                         in1=rb[:, :])
                    # place into the pair tile (DMA shifts partitions)
                    t, half = h // 2, (h % 2) * 64
                    nc.sync.dma_start(out=attnT[t][half:half + 64, :],
                                      in_=ah[:, :])

        # =============== phase E: output projection + bias ================
        with tc.tile_pool(name="wop", bufs=6) as wop, \
             tc.tile_pool(name="yps", bufs=4, space="PSUM") as yps, \
             tc.tile_pool(name="ystg", bufs=3) as ystg:
            for nt in range(4):            # 4 output column blocks of 512
                wo_last = wop.tile([1, 512], BF, tag="wolast")
                nc.sync.dma_start(out=wo_last[:, :],
                                  in_=woT[HID:HID + 1, nt * 512:(nt + 1) * 512])
                pys = [yps.tile([128, 512], F32, tag="py", name=f"py{nt}_{i}")
                       for i in range(4)]
                for kc in range(KC):
                    wo_t = wop.tile([128, 512], BF, tag="wo")
                    nc.sync.dma_start(
                        out=wo_t[:, :],
                        in_=woT[kc * 128:(kc + 1) * 128, nt * 512:(nt + 1) * 512])
                    for mt in range(4):    # 4 token blocks of 128
                        nc.tensor.matmul(
                            out=pys[mt][:, :],
                            lhsT=attnT[kc][:, mt * 128:(mt + 1) * 128],
                            rhs=wo_t[:, :],
                            start=(kc == 0), stop=False)
                for mt in range(4):        # bias via ones row, K=1 matmul
                    nc.tensor.matmul(
                        out=pys[mt][:, :], lhsT=ones128[:, :],
                        rhs=wo_last[:, :], start=False, stop=True)
                    ys = ystg.tile([128, 512], F32, tag="ys")
                    nc.vector.tensor_copy(out=ys[:, :], in_=pys[mt][:, :])
                    nc.sync.dma_start(
                        out=out[mt * 128:(mt + 1) * 128, nt * 512:(nt + 1) * 512],
                        in_=ys[:, :])

    nc.finalize()
    return nc


@functools.lru_cache(maxsize=1)
def _graph():
    return build_graph()


def make_in_maps(x, Wq, Wk, Wv, Wo, bo):
    bf16 = ml_dtypes.bfloat16
    x = np.asarray(x, np.float32)
    wqT = np.ascontiguousarray(np.asarray(Wq, np.float32).T).astype(bf16)
    wkT = np.ascontiguousarray(np.asarray(Wk, np.float32).T).astype(bf16)
    wvT = np.ascontiguousarray(np.asarray(Wv, np.float32).T).astype(bf16)
    woT = np.concatenate(
        [np.asarray(Wo, np.float32).T,
         np.asarray(bo, np.float32)[None, :]], axis=0).astype(bf16)
    woT = np.ascontiguousarray(woT)
    in_maps = []
    for c in range(8):
        b, r = c // TP, c % TP
        xT_c = np.ascontiguousarray(
            x[b].T[:, r * TOK:(r + 1) * TOK]).astype(bf16)
        in_maps.append(
            {"xT": xT_c, "wqT": wqT, "wkT": wkT, "wvT": wvT, "woT": woT})
    return in_maps


def kernel(x, Wq, Wk, Wv, Wo, bo):
    nc = _graph()
    in_maps = make_in_maps(x, Wq, Wk, Wv, Wo, bo)
    res = run_bass_kernel_spmd(nc, in_maps, core_ids=list(range(8)))
    out = np.empty((B, S, HID), np.float32)
    for c in range(8):
        b, r = c // TP, c % TP
        out[b, r * TOK:(r + 1) * TOK, :] = np.asarray(
            res.results[c]["out"], np.float32)
    return out
